# revision 1
# baseline (speedup 1.0000x reference)
"""Trainium2 Bass kernel for nn_ClassificationModel (CNN window encoder +
4-layer post-norm transformer + mean-pool classifier head).

Data parallel across 8 NeuronCores: batch N=64 -> 8 samples/core.

v2: restructured for engine balance / fewer ops than the original version:
  - Q/K projections batched across all 8 samples (N=512 matmuls).
  - Scores computed transposed (S^T = K^T·Q per head) so exp(S^T) feeds the
    A·V matmul directly -- no per-head PE transpose + copy.
  - Softmax without max-subtraction (logits are tiny); row sums come free
    from a ones-column appended to V.
  - All biases (V, Wo, W2, embed) folded into the matmuls as K=1 rank-1
    updates; residual adds folded in via identity-weight matmuls.
  - LayerNorm stats from ACT accum_out (sum, sum-of-squares); the per-row
    rsqrt is batched into one ACT op per sublayer to avoid activation-table
    thrash (exp and sqrt live in different ACT tables).
  - Conv max-pooling via cross-partition-base scalar_tensor_tensor reading
    PSUM directly (relu(max(lo+b, hi+b)) == max(lo+b, relu(hi+b))) -- no
    SBUF->SBUF shift DMAs.
"""

import math
import sys

sys.path.insert(0, "/opt/trn_rl_repo")

import numpy as np
import ml_dtypes

import concourse.bass as bass
import concourse.mybir as mybir
import concourse.tile as tile
from concourse import bacc
from concourse.bass import AP
from concourse.bass_utils import run_bass_kernel_spmd

BF = ml_dtypes.bfloat16
F32 = mybir.dt.float32
BF16 = mybir.dt.bfloat16
AX = mybir.AxisListType
OP = mybir.AluOpType
AF = mybir.ActivationFunctionType

# model dims
N, L, W = 64, 128, 256
D, H, NL, DFF = 384, 8, 4, 1536
E = D // H  # 48
CH = [1, 4, 16, 64]
K = 7
NCORES = 8
RPC = N // NCORES          # samples per core = 8
R = RPC * L                # rows per core = 1024
TEMP = 1.0 / math.sqrt(E)
EPS = 1e-5

# conv block sizes (output positions per Toeplitz block)
B0, B1, B2 = 32, 8, 2
NB0, NB1, NB2 = 256 // B0, 128 // B1, 64 // B2  # 8, 16, 32


# ---------------------------------------------------------------------------
# host-side weight preparation
# ---------------------------------------------------------------------------

def _pe_np(l, d):
    pos = np.arange(l)[:, None].astype(np.float32)
    i = np.arange(d // 2)[None, :].astype(np.float32)
    ang = pos / np.power(10000.0, 2.0 * i / d)
    pe = np.zeros((l, d), np.float32)
    pe[:, 0::2] = np.sin(ang)
    pe[:, 1::2] = np.cos(ang)
    return pe


# conv source-block overlap enumeration (shared host/device) -----------------

# (Bout, src_size, nsrc, nch): conv0 reads xT tiles (128 pos each);
# conv1 reads pooled0 blocks (16 pos, 4 ch); conv2 reads pooled1 (4 pos, 16 ch)
CONV_GEOM = {
    0: (B0, 128, 2, 1),
    1: (B1, 16, NB0, 4),
    2: (B2, 4, NB1, 16),
}


def overlaps(conv, b):
    """source tiles overlapping output block b's input window; (src, delta)."""
    Bout, src_size, nsrc, _ = CONV_GEOM[conv]
    w0, w1 = Bout * b - 3, Bout * b + Bout + 3
    res = []
    for s in range(nsrc):
        lo, hi = s * src_size, (s + 1) * src_size
        if max(w0, lo) < min(w1, hi):
            res.append((s, lo - Bout * b))
    return res


def conv_deltas(conv):
    nb = {0: NB0, 1: NB1, 2: NB2}[conv]
    ds = sorted({d for b in range(nb) for _, d in overlaps(conv, b)})
    return ds


def _m_layout(conv, h, co):
    if conv == 0:
        return (h & 1) * 64 + (h >> 1) * 4 + co
    if conv == 1:
        return (h & 1) * 64 + (h >> 1) * 16 + co
    return h * 64 + co


def _toeplitz_variants(conv, w):
    """w: (C_out, C_in, K). returns (nvar, src_size*nch, 128) f32."""
    Bout, src_size, _, nch = CONV_GEOM[conv]
    cout = w.shape[0]
    ds = conv_deltas(conv)
    T = np.zeros((len(ds), src_size * nch, 128), np.float32)
    for vi, delta in enumerate(ds):
        for hp in range(src_size):
            for h in range(Bout):
                k = delta + hp - h + 3
                if 0 <= k < K:
                    for co in range(cout):
                        for ci in range(nch):
                            T[vi, hp * nch + ci, _m_layout(conv, h, co)] = w[co, ci, k]
    return T


def host_prep(inp):
    d = {}
    f32 = np.float32
    d["T0"] = _toeplitz_variants(0, np.asarray(inp["conv_w0"], f32)).astype(BF)
    d["T1"] = _toeplitz_variants(1, np.asarray(inp["conv_w1"], f32)).astype(BF)
    d["T2"] = _toeplitz_variants(2, np.asarray(inp["conv_w2"], f32)).astype(BF)
    b0, b1, b2 = (np.asarray(inp[f"conv_b{i}"], f32) for i in range(3))
    p = np.arange(128)
    d["b0e"] = b0[p % 4].reshape(128, 1)
    d["b1e"] = b1[p % 16].reshape(128, 1)
    d["b2e"] = b2[p % 64].reshape(128, 1)

    # embed: We_r[c, p, :] = embed_w[(p%64)*32 + 2c + p//64, :]
    ew = np.asarray(inp["embed_w"], f32)  # (2048, 384)
    We_r = np.zeros((16, 128, D), f32)
    for c in range(16):
        for pi in range(128):
            We_r[c, pi] = ew[(pi % 64) * 32 + 2 * c + pi // 64]
    d["We_r"] = We_r.astype(BF)
    d["eb_row"] = np.asarray(inp["embed_b"], f32).reshape(1, D).astype(BF)
    d["pe_rm"] = _pe_np(L, D)

    # Q/K projected compact (feature-major, unpadded); heads are spread to
    # 64-row zero-padded slots afterwards by SBUF->SBUF DMA (partition remap).
    for nm in ("Wq", "Wk"):
        d[nm] = np.asarray(inp[nm], f32).astype(BF)  # (4, 384, 384)
    for nm in ("Wv", "Wo"):
        d[nm] = np.asarray(inp[nm], f32).astype(BF)  # (4, 384, 384)
    d["W1"] = np.asarray(inp["W1"], f32).astype(BF)  # (4, 384, 1536)
    d["W2"] = np.asarray(inp["W2"], f32).astype(BF)  # (4, 1536, 384)

    def _qk_bias(b):  # (4, 384) -> (4, 128, 3): per output-chunk columns
        out = np.zeros((NL, 128, 3), f32)
        for c in range(3):
            out[:, :, c] = b[:, c * 128:(c + 1) * 128]
        return out
    d["bq_q"] = _qk_bias(np.asarray(inp["bq"], f32))
    d["bk_q"] = _qk_bias(np.asarray(inp["bk"], f32))
    d["bv_row"] = np.asarray(inp["bv"], f32).reshape(NL, 1, D).astype(BF)
    d["bo_row"] = np.asarray(inp["bo"], f32).reshape(NL, 1, D).astype(BF)
    d["b2_row"] = np.asarray(inp["b2"], f32).reshape(NL, 1, D).astype(BF)
    for nm, src in (("g1_b", "g1"), ("be1_b", "be1"), ("g2_b", "g2"), ("be2_b", "be2")):
        a = np.asarray(inp[src], f32)  # (4, 384)
        d[nm] = np.broadcast_to(a[:, None, :], (NL, 128, D)).astype(BF).copy()
    b1f = np.asarray(inp["b1"], f32)  # (4, 1536)
    d["b1_r"] = np.stack([b1f[l].reshape(12, 128).T for l in range(NL)])  # (4,128,12)

    d["idn_f"] = np.eye(128, dtype=f32)
    d["idn_b"] = np.eye(128, dtype=f32).astype(BF)
    d["onesL"] = np.full((128, 1), 1.0 / L, f32)
    d["clsw_r"] = np.asarray(inp["cls_w"], f32).reshape(3, 128).T.copy()  # (128,3)
    d["clsb"] = np.asarray(inp["cls_b"], f32).reshape(1, 1)
    d["epsc"] = np.full((128, 1), EPS, f32)
    return d


# ---------------------------------------------------------------------------
# device program
# ---------------------------------------------------------------------------

def build_program(debug=None, do_compile=True, n_layers=NL, phase=99):
    nc = bacc.Bacc("TRN2", target_bir_lowering=False, debug=False)

    di = {}  # dram inputs
    def dram_in(name, shape, dt=BF16):
        di[name] = nc.dram_tensor(name, list(shape), dt, kind="ExternalInput")
        return di[name]

    x_d = dram_in("xc", (R, W), F32)
    nv0, nv1, nv2 = len(conv_deltas(0)), len(conv_deltas(1)), len(conv_deltas(2))
    T0_d = dram_in("T0", (nv0, 128, 128))
    T1_d = dram_in("T1", (nv1, 64, 128))
    T2_d = dram_in("T2", (nv2, 64, 128))
    b0e_d = dram_in("b0e", (128, 1), F32)
    b1e_d = dram_in("b1e", (128, 1), F32)
    b2e_d = dram_in("b2e", (128, 1), F32)
    We_d = dram_in("We_r", (16, 128, D))
    ebr_d = dram_in("eb_row", (1, D))
    pe_d = dram_in("pe_rm", (128, D), F32)
    wq_d = dram_in("Wq", (NL, D, D))
    wk_d = dram_in("Wk", (NL, D, D))
    wv_d = dram_in("Wv", (NL, D, D))
    wo_d = dram_in("Wo", (NL, D, D))
    w1_d = dram_in("W1", (NL, D, DFF))
    w2_d = dram_in("W2", (NL, DFF, D))
    bqq_d = dram_in("bq_q", (NL, 128, 3), F32)
    bkq_d = dram_in("bk_q", (NL, 128, 3), F32)
    bvr_d = dram_in("bv_row", (NL, 1, D))
    bor_d = dram_in("bo_row", (NL, 1, D))
    b2r_d = dram_in("b2_row", (NL, 1, D))
    g1_d = dram_in("g1_b", (NL, 128, D))
    be1_d = dram_in("be1_b", (NL, 128, D))
    g2_d = dram_in("g2_b", (NL, 128, D))
    be2_d = dram_in("be2_b", (NL, 128, D))
    b1r_d = dram_in("b1_r", (NL, 128, 12), F32)
    idnf_d = dram_in("idn_f", (128, 128), F32)
    idnb_d = dram_in("idn_b", (128, 128))
    onesL_d = dram_in("onesL", (128, 1), F32)
    clsw_d = dram_in("clsw_r", (128, 3), F32)
    eps_d = dram_in("epsc", (128, 1), F32)
    clsb_d = dram_in("clsb", (1, 1), F32)

    y_d = nc.dram_tensor("yc", [RPC, 1], F32, kind="ExternalOutput")
    dbg_d = None
    if debug is not None:
        dbg_d = nc.dram_tensor("dbg", [R, D], F32, kind="ExternalOutput")

    from contextlib import ExitStack
    with tile.TileContext(nc) as tc, ExitStack() as ctx:
        const = ctx.enter_context(tc.tile_pool(name="const", bufs=1))
        state = ctx.enter_context(tc.tile_pool(name="state", bufs=1))
        psA = ctx.enter_context(tc.tile_pool(name="psA", bufs=3, space="PSUM"))
        psB = ctx.enter_context(tc.tile_pool(name="psB", bufs=3, space="PSUM"))
        psC = ctx.enter_context(tc.tile_pool(name="psC", bufs=2, space="PSUM"))

        xin = ctx.enter_context(tc.tile_pool(name="xin", bufs=1))
        x_ts = []
        for rt in range(RPC):
            t = xin.tile([128, W], F32, tag=f"x_t{rt}", name=f"x_t{rt}")
            nc.sync.dma_start(t[:], x_d[rt * 128:(rt + 1) * 128, :])
            x_ts.append(t)

        def load_const(dram, shape, dt):
            nm = dram.name + "_sb"
            t = const.tile(list(shape), dt, tag=nm, name=nm)
            nc.sync.dma_start(t[:], dram[:])
            return t

        T0v, T1v, T2v = [], [], []
        for conv, (dst, dram, npart) in enumerate(
                ((T0v, T0_d, 128), (T1v, T1_d, 64), (T2v, T2_d, 64))):
            for vi in range(len(conv_deltas(conv))):
                t = const.tile([npart, 128], BF16, tag=f"Tv{conv}_{vi}",
                               name=f"Tv{conv}_{vi}")
                nc.sync.dma_start(t[:], dram[vi])
                dst.append(t)
        d2i = [{d: i for i, d in enumerate(conv_deltas(c))} for c in range(3)]
        b0e = load_const(b0e_d, (128, 1), F32)
        b1e = load_const(b1e_d, (128, 1), F32)
        b2e = load_const(b2e_d, (128, 1), F32)
        eb_row = load_const(ebr_d, (1, D), BF16)
        pe_rm = load_const(pe_d, (128, D), F32)
        idn_f = load_const(idnf_d, (128, 128), F32)
        idn_b = load_const(idnb_d, (128, 128), BF16)
        onesL = load_const(onesL_d, (128, 1), F32)
        clsw = load_const(clsw_d, (128, 3), F32)
        epsc = load_const(eps_d, (128, 1), F32)
        clsb = load_const(clsb_d, (1, 1), F32)
        We = []
        for c in range(16):
            t = const.tile([128, D], BF16, tag=f"We{c}", name=f"We{c}")
            nc.sync.dma_start(t[:], We_d[c])
            We.append(t)
        ones_bf = const.tile([1, 512], BF16, tag="ones_bf", name="ones_bf")
        nc.vector.memset(ones_bf[:], 1.0)

        # persistent state
        t_rm = [state.tile([128, D], F32, tag=f"t_rm{rt}", name=f"t_rm{rt}") for rt in range(RPC)]
        t_fm = state.tile([128, 3, R], BF16, tag="t_fm", name="t_fm")
        o_fm = state.tile([128, 3, R], BF16, tag="o_fm", name="o_fm")
        h1 = [state.tile([128, R], BF16, tag=f"h1_{c}", name=f"h1_{c}") for c in range(12)]

        # ------------------------------------------------------- CNN + embed
        with tc.tile_pool(name="cnn", bufs=3) as cnnp:
            for rt in range(RPC):
                x_t = x_ts[rt]

                xT = []
                for half in range(2):
                    ps = psC.tile([128, 128], F32, tag="psC", name="psC")
                    nc.tensor.transpose(ps[:], x_t[:, half * 128:(half + 1) * 128], idn_f[:])
                    xt = cnnp.tile([128, 128], BF16, tag=f"xT{half}", name=f"xT{half}")
                    nc.scalar.copy(xt[:], ps[:])
                    xT.append(xt)

                def conv_stage(conv, Tv, srcs, bias, ngroups, out_pooled):
                    # pooled[:, g*4:(g+1)*4, :] <- relu(max(lo,hi)+b) via
                    # max(lo+b, relu(hi+b)) == max(lo+b, hi+b, 0)
                    for g in range(ngroups):
                        ps = psA.tile([128, 512], F32, tag="psA", name="psA")
                        for bb in range(4):
                            b = g * 4 + bb
                            ovl = overlaps(conv, b)
                            for i, (s, dlt) in enumerate(ovl):
                                nc.tensor.matmul(
                                    ps[:, bb * 128:(bb + 1) * 128],
                                    lhsT=Tv[d2i[conv][dlt]][:], rhs=srcs(s),
                                    start=(i == 0), stop=(i == len(ovl) - 1))
                        r_hi = cnnp.tile([64, 512], BF16, tag="r_hi", name="r_hi")
                        nc.scalar.activation(r_hi[:], ps[64:128, :], AF.Relu,
                                             bias=bias[64:128, :])
                        nc.vector.scalar_tensor_tensor(
                            out_pooled[:, g * 4:(g + 1) * 4, :],
                            in0=ps[0:64, :].rearrange("p (b r) -> p b r", b=4),
                            scalar=bias[0:64, :],
                            in1=r_hi[:].rearrange("p (b r) -> p b r", b=4),
                            op0=OP.add, op1=OP.max)

                # conv0 -> pooled0 (64 = hp*4+co, 8 blocks, 128 rows)
                pooled0 = cnnp.tile([64, NB0, 128], BF16, tag="pooled0", name="pooled0")
                conv_stage(0, T0v, lambda s: xT[s][:], b0e, 2, pooled0)

                # conv1 -> pooled1 (64 = hp*16+co, 16 blocks, 128 rows)
                pooled1 = cnnp.tile([64, NB1, 128], BF16, tag="pooled1", name="pooled1")
                conv_stage(1, T1v, lambda s: pooled0[:, s, :], b1e, 4, pooled1)

                # conv2 -> act3 (128 = (b&1)*64+co, 16 chunks, 128 rows)
                act3 = cnnp.tile([128, 16, 128], BF16, tag="act3", name="act3")
                for g in range(8):
                    ps = psA.tile([128, 512], F32, tag="psA", name="psA")
                    for bb in range(4):
                        b = g * 4 + bb
                        ovl = overlaps(2, b)
                        for i, (s, dlt) in enumerate(ovl):
                            nc.tensor.matmul(
                                ps[:, bb * 128:(bb + 1) * 128],
                                lhsT=T2v[d2i[2][dlt]][:], rhs=pooled1[:, s, :],
                                start=(i == 0), stop=(i == len(ovl) - 1))
                    r_hi = cnnp.tile([64, 512], BF16, tag="r_hi", name="r_hi")
                    nc.scalar.activation(r_hi[:], ps[64:128, :], AF.Relu,
                                         bias=b2e[64:128, :])
                    # blocks g*4 .. g*4+3: block = 2*b + e; even e ->
                    # partitions 0:64, odd e -> 64:128; chunks 2g, 2g+1
                    lo_v = ps[0:64, :].rearrange("p (b e r) -> p e b r", b=2, e=2)
                    hi_v = r_hi[:].rearrange("p (b e r) -> p e b r", b=2, e=2)
                    nc.vector.scalar_tensor_tensor(
                        act3[0:64, 2 * g:2 * g + 2, :],
                        in0=lo_v[:, 0], scalar=b2e[0:64, :], in1=hi_v[:, 0],
                        op0=OP.add, op1=OP.max)
                    nc.vector.scalar_tensor_tensor(
                        act3[64:128, 2 * g:2 * g + 2, :],
                        in0=lo_v[:, 1], scalar=b2e[0:64, :], in1=hi_v[:, 1],
                        op0=OP.add, op1=OP.max)

                # embed (row-major out) + bias fold + relu + pe
                pse = psB.tile([128, 392], F32, tag="psB", name="psB")
                for c in range(16):
                    nc.tensor.matmul(pse[:, 0:D], lhsT=act3[:, c, :], rhs=We[c][:],
                                     start=(c == 0), stop=False)
                nc.tensor.matmul(pse[:, 0:D], lhsT=ones_bf[:, 0:128], rhs=eb_row[:],
                                 start=False, stop=True)
                er = cnnp.tile([128, D], F32, tag="er", name="er")
                nc.scalar.activation(er[:], pse[:, 0:D], AF.Relu)
                nc.vector.tensor_tensor(t_rm[rt][:], er[:], pe_rm[:], OP.add)

        # ------------------------------------------------------- transformer
        # early-in-layer weights double-buffered (cross-layer prefetch);
        # late-used ones single-buffered (their slot frees mid-layer anyway)
        wpool2 = ctx.enter_context(tc.tile_pool(name="wpool2", bufs=2))
        wpool = ctx.enter_context(tc.tile_pool(name="wpool", bufs=1))
        qkp = ctx.enter_context(tc.tile_pool(name="qkp", bufs=1))
        xpool = ctx.enter_context(tc.tile_pool(name="xpool", bufs=1))
        work = ctx.enter_context(tc.tile_pool(name="work", bufs=3))
        qc_t = qkp.tile([128, 3, R], BF16, tag="qc_t", name="qc_t")
        kc_t = qkp.tile([128, 3, R], BF16, tag="kc_t", name="kc_t")
        qf_all = qkp.tile([64, H, R], BF16, tag="qf_all", name="qf_all")
        kf_all = qkp.tile([64, H, R], BF16, tag="kf_all", name="kf_all")
        nc.vector.memset(qf_all[32:64, :, :], 0.0)
        nc.vector.memset(kf_all[32:64, :, :], 0.0)

        for lyr in range(n_layers):
            wq = [wpool2.tile([128, D], BF16, tag=f"wq{c}", name=f"wq{c}") for c in range(3)]
            wk = [wpool2.tile([128, D], BF16, tag=f"wk{c}", name=f"wk{c}") for c in range(3)]
            wv = [wpool2.tile([128, D], BF16, tag=f"wv{c}", name=f"wv{c}") for c in range(3)]
            wo = [wpool.tile([128, D], BF16, tag=f"wo{c}", name=f"wo{c}") for c in range(3)]
            w1 = [wpool.tile([128, DFF], BF16, tag=f"w1{c}", name=f"w1{c}") for c in range(3)]
            w2 = [wpool.tile([128, D], BF16, tag=f"w2{c}", name=f"w2{c}") for c in range(12)]
            for c in range(3):
                nc.sync.dma_start(wq[c][:], wq_d[lyr, c * 128:(c + 1) * 128, :])
                nc.sync.dma_start(wk[c][:], wk_d[lyr, c * 128:(c + 1) * 128, :])
                nc.sync.dma_start(wv[c][:], wv_d[lyr, c * 128:(c + 1) * 128, :])
                nc.sync.dma_start(wo[c][:], wo_d[lyr, c * 128:(c + 1) * 128, :])
                nc.sync.dma_start(w1[c][:], w1_d[lyr, c * 128:(c + 1) * 128, :])
            for c in range(12):
                nc.sync.dma_start(w2[c][:], w2_d[lyr, c * 128:(c + 1) * 128, :])
            bqq = wpool2.tile([128, 3], F32, tag="bqq", name="bqq")
            bkq = wpool2.tile([128, 3], F32, tag="bkq", name="bkq")
            nc.sync.dma_start(bqq[:], bqq_d[lyr])
            nc.sync.dma_start(bkq[:], bkq_d[lyr])
            lb = {}
            for nm, dd, shp in (("bvr", bvr_d, (1, D)), ("bor", bor_d, (1, D)),
                                ("b2r", b2r_d, (1, D)), ("g1", g1_d, (128, D)),
                                ("be1", be1_d, (128, D)), ("g2", g2_d, (128, D)),
                                ("be2", be2_d, (128, D))):
                lb[nm] = wpool.tile(list(shp), BF16, tag=f"lb_{nm}", name=f"lb_{nm}")
                nc.sync.dma_start(lb[nm][:], dd[lyr])
            b1r = wpool.tile([128, 12], F32, tag="b1r", name="b1r")
            nc.sync.dma_start(b1r[:], b1r_d[lyr])

            # t_fm <- transpose(t_rm): 3 chunk transposes -> one wide copy
            for rt in range(RPC):
                ps = psC.tile([128, 3, 128], F32, tag="psC", name="psC")
                for c in range(3):
                    nc.tensor.transpose(ps[:, c, :], t_rm[rt][:, c * 128:(c + 1) * 128], idn_f[:])
                nc.vector.tensor_copy(t_fm[:, :, rt * 128:(rt + 1) * 128], ps[:])

            if phase < 2:
                continue
            # batched Q/K, compact feature-major: 3 out-chunks x 2 halves
            for dstc, wmat, brow in ((qc_t, wq, bqq), (kc_t, wk, bkq)):
                for oc in range(3):
                    for hf in range(2):
                        pq = psA.tile([128, 512], F32, tag="psA", name="psA")
                        for c in range(3):
                            nc.tensor.matmul(pq[:], lhsT=wmat[c][:, oc * 128:(oc + 1) * 128],
                                             rhs=t_fm[:, c, hf * 512:(hf + 1) * 512],
                                             start=(c == 0), stop=(c == 2))
                        nc.scalar.activation(dstc[:, oc, hf * 512:(hf + 1) * 512],
                                             pq[:], AF.Identity, bias=brow[:, oc:oc + 1])
            # spread head rows 48h..48h+47 into 64-row zero-padded slots
            for dst64, srcc in ((qf_all, qc_t), (kf_all, kc_t)):
                for h in range(H):
                    dr, r = 0, 48 * h
                    while dr < 48:
                        c0, r0 = r // 128, r % 128
                        n = min(48 - dr, 128 - r0)
                        nc.gpsimd.dma_start(dst64[dr:dr + n, h, :],
                                            srcc[r0:r0 + n, c0, :])
                        dr += n
                        r += n

            # per-sample attention
            for n in range(RPC):
                cs = slice(n * 128, (n + 1) * 128)
                if phase < 3:
                    break
                # V (+ bias fold) -> v_ext with ones column per head
                pv = psB.tile([128, 392], F32, tag="psB", name="psB")
                for c in range(3):
                    nc.tensor.matmul(pv[:, 0:D], lhsT=t_fm[:, c, cs], rhs=wv[c][:],
                                     start=(c == 0), stop=False)
                nc.tensor.matmul(pv[:, 0:D], lhsT=ones_bf[:, 0:128], rhs=lb["bvr"][:],
                                 start=False, stop=True)
                v_ext = work.tile([128, H, E + 1], BF16, tag="v_ext", name="v_ext")
                nc.vector.tensor_copy(
                    v_ext[:, :, 0:E],
                    pv[:, 0:D].rearrange("p (h e) -> p h e", h=H))
                nc.vector.memset(v_ext[:, :, E:E + 1], 1.0)
                if phase < 4:
                    continue

                # scores S^T = K^T Q per head; exp with scale, no max-sub
                es16 = work.tile([128, H, 128], BF16, tag="es16", name="es16")
                for hf in range(2):
                    pss = psA.tile([128, 512], F32, tag="psA", name="psA")
                    for hh in range(4):
                        h = hf * 4 + hh
                        nc.tensor.matmul(
                            pss[:, hh * 128:(hh + 1) * 128],
                            lhsT=kf_all[:, h, cs], rhs=qf_all[:, h, cs],
                            start=True, stop=True)
                    nc.scalar.activation(
                        es16[:, hf * 4:(hf + 1) * 4, :],
                        pss[:].rearrange("p (a b) -> p a b", a=4),
                        AF.Exp, scale=TEMP)
                if phase < 5:
                    continue

                # A·[V|1] -> per-head 49-col groups: o unnormalized + row sums
                pso = psB.tile([128, 392], F32, tag="psB", name="psO")
                for h in range(H):
                    nc.tensor.matmul(pso[:, h * 49:(h + 1) * 49],
                                     lhsT=es16[:, h, :], rhs=v_ext[:, h, :],
                                     start=True, stop=True)
                pso_v = pso[:, 0:392].rearrange("p (h e) -> p h e", h=H)
                rr = work.tile([128, H], F32, tag="rr", name="rr")
                nc.vector.reciprocal(rr[:], pso_v[:, :, E])
                o_rm = work.tile([128, D], BF16, tag="o_rm", name="o_rm")
                rrb = AP(rr.tensor, rr.offset, [list(rr.ap[0]), [1, H], [0, E]])
                nc.vector.tensor_tensor(o_rm[:].rearrange("p (h e) -> p h e", h=H),
                                        pso_v[:, :, 0:E], rrb, OP.mult)
                ps = psC.tile([128, 3, 128], BF16, tag="psC", name="psC")
                for c in range(3):
                    nc.tensor.transpose(ps[:, c, :], o_rm[:, c * 128:(c + 1) * 128], idn_b[:])
                nc.vector.tensor_copy(o_fm[:, :, cs], ps[:])

            # u = o @ Wo + bo + t (residual via identity matmul); LN1 -> t_rm
            def ln_sublayer(x_src_mm, gb, beb, relu_dc=None):
                """x_src_mm(rt, psum) emits matmuls accumulating the full
                pre-LN activation (incl. residual + bias) into psum[:, 0:D].
                Then: stats via ACT accum, batched rsqrt, affine, g/be."""
                HB = RPC // 2
                for hb in range(2):
                    rts = range(hb * HB, (hb + 1) * HB)
                    x1s = {}
                    vpack = work.tile([128, HB], F32, tag="vpack", name="vpack")
                    meanp = work.tile([128, HB], F32, tag="meanp", name="meanp")
                    for rt in rts:
                        j = rt - hb * HB
                        px = psB.tile([128, 392], F32, tag="psB", name="psB")
                        x_src_mm(rt, px)
                        x1 = xpool.tile([128, D], F32, tag=f"x1_{rt}", name=f"x1_{rt}")
                        s1 = work.tile([128, 1], F32, tag="s1", name="s1")
                        # x1 = px + t_rm (residual) with free running sum
                        nc.vector.scalar_tensor_tensor(
                            x1[:], in0=px[:, 0:D], scalar=0.0, in1=t_rm[rt][:],
                            op0=OP.add, op1=OP.add, accum_out=s1[:])
                        xsq = work.tile([128, D], BF16, tag="xsq", name="xsq")
                        s2 = work.tile([128, 1], F32, tag="s2", name="s2")
                        nc.vector.scalar_tensor_tensor(
                            xsq[:], in0=x1[:], scalar=0.0, in1=x1[:],
                            op0=OP.add, op1=OP.mult, accum_out=s2[:])
                        # mean, var -> packed cols
                        nc.vector.tensor_scalar(meanp[:, j:j + 1], s1[:], 1.0 / D,
                                                None, OP.mult)
                        msq = work.tile([128, 1], F32, tag="msq", name="msq")
                        nc.vector.tensor_tensor(msq[:], meanp[:, j:j + 1],
                                                meanp[:, j:j + 1], OP.mult)
                        nc.vector.scalar_tensor_tensor(
                            vpack[:, j:j + 1], in0=s2[:], scalar=1.0 / D,
                            in1=msq[:], op0=OP.mult, op1=OP.subtract)
                        x1s[rt] = x1
                    sdp = work.tile([128, HB], F32, tag="sdp", name="sdp")
                    nc.scalar.activation(sdp[:], vpack[:], AF.Sqrt, bias=epsc[:])
                    rstdp = work.tile([128, HB], F32, tag="rstdp", name="rstdp")
                    nc.vector.reciprocal(rstdp[:], sdp[:])
                    for rt in rts:
                        j = rt - hb * HB
                        mrs = work.tile([128, 1], F32, tag="mrs", name="mrs")
                        nc.vector.scalar_tensor_tensor(
                            mrs[:], in0=meanp[:, j:j + 1], scalar=-1.0,
                            in1=rstdp[:, j:j + 1], op0=OP.mult, op1=OP.mult)
                        xn = work.tile([128, D], F32, tag="xn", name="xn")
                        nc.scalar.activation(xn[:], x1s[rt][:], AF.Identity,
                                             bias=mrs[:], scale=rstdp[:, j:j + 1])
                        xg = work.tile([128, D], F32, tag="xg", name="xg")
                        nc.gpsimd.tensor_tensor(xg[:], xn[:], gb[:], OP.mult)
                        nc.gpsimd.tensor_tensor(t_rm[rt][:], xg[:], beb[:], OP.add)

            if phase < 6:
                continue

            def wo_mm(rt, px):
                cs = slice(rt * 128, (rt + 1) * 128)
                for c in range(3):
                    nc.tensor.matmul(px[:, 0:D], lhsT=o_fm[:, c, cs], rhs=wo[c][:],
                                     start=(c == 0), stop=False)
                nc.tensor.matmul(px[:, 0:D], lhsT=ones_bf[:, 0:128], rhs=lb["bor"][:],
                                 start=False, stop=True)

            ln_sublayer(wo_mm, lb["g1"], lb["be1"])

            if phase < 7:
                continue
            # FFN
            for rt in range(RPC):
                ps = psC.tile([128, 3, 128], F32, tag="psC", name="psC")
                for c in range(3):
                    nc.tensor.transpose(ps[:, c, :], t_rm[rt][:, c * 128:(c + 1) * 128], idn_f[:])
                nc.vector.tensor_copy(t_fm[:, :, rt * 128:(rt + 1) * 128], ps[:])
            for dc in range(12):
                for nh in range(2):
                    ph = psA.tile([128, 512], F32, tag="psA", name="psA")
                    for c in range(3):
                        nc.tensor.matmul(ph[:], lhsT=w1[c][:, dc * 128:(dc + 1) * 128],
                                         rhs=t_fm[:, c, nh * 512:(nh + 1) * 512],
                                         start=(c == 0), stop=(c == 2))
                    nc.scalar.activation(h1[dc][:, nh * 512:(nh + 1) * 512], ph[:],
                                         AF.Relu, bias=b1r[:, dc:dc + 1])
            if phase < 8:
                continue

            def w2_mm(rt, px):
                cs = slice(rt * 128, (rt + 1) * 128)
                for dc in range(12):
                    nc.tensor.matmul(px[:, 0:D], lhsT=h1[dc][:, cs], rhs=w2[dc][:],
                                     start=(dc == 0), stop=False)
                nc.tensor.matmul(px[:, 0:D], lhsT=ones_bf[:, 0:128], rhs=lb["b2r"][:],
                                 start=False, stop=True)

            ln_sublayer(w2_mm, lb["g2"], lb["be2"])

        if dbg_d is not None:
            for rt in range(RPC):
                nc.sync.dma_start(dbg_d[rt * 128:(rt + 1) * 128, :], t_rm[rt][:])

        # ------------------------------------------------------- head
        outsb = state.tile([1, RPC], F32, tag="outsb", name="outsb")
        for n in range(RPC):
            pm = psC.tile([128, 3], F32, tag="psC", name="psCh")
            for c in range(3):
                nc.tensor.matmul(pm[:, c:c + 1], lhsT=t_rm[n][:, c * 128:(c + 1) * 128],
                                 rhs=onesL[:], start=True, stop=True)
            tm = work.tile([128, 3], F32, tag="tm", name="tm")
            nc.scalar.copy(tm[:], pm[:])
            pc = psC.tile([1, 8], F32, tag="psC", name="psCh2")
            for c in range(3):
                nc.tensor.matmul(pc[:, 0:1], lhsT=tm[:, c:c + 1], rhs=clsw[:, c:c + 1],
                                 start=(c == 0), stop=(c == 2))
            nc.scalar.activation(outsb[:, n:n + 1], pc[:, 0:1], AF.Identity,
                                 bias=clsb[:])
        nc.sync.dma_start(y_d[:].rearrange("a b -> b a"), outsb[:])

    if do_compile:
        nc.compile()
    return nc


_PROG = {}


def _get_prog(debug=None, n_layers=NL, phase=99):
    key = ("dbg" if debug else "plain", n_layers, phase)
    if key not in _PROG:
        _PROG[key] = build_program(debug, n_layers=n_layers, phase=phase)
    return _PROG[key]


def _in_maps(inputs):
    shared = host_prep(inputs)
    x = np.asarray(inputs["x"], np.float32)  # (64, 128, 256)
    in_maps = []
    for c in range(NCORES):
        m = dict(shared)
        m["xc"] = np.ascontiguousarray(
            x[c * RPC:(c + 1) * RPC].reshape(R, W))
        in_maps.append(m)
    return in_maps


def kernel(**inputs):
    nc = _get_prog()
    res = run_bass_kernel_spmd(nc, _in_maps(inputs), core_ids=list(range(NCORES)))
    out = np.concatenate([res.results[c]["yc"] for c in range(NCORES)], axis=0)
    return out.astype(np.float32)


def timed_run(inputs, iters=32):
    """Estimate per-execution HW time by chaining NEFF executions.

    No NTFF hook is available through this axon tunnel, so true HW exec
    time can't be read from a profile.  Instead we chain k executions
    (each iteration's outputs feed the next call's operands, forcing
    device-side serialization while dispatch pipelines) and report the
    marginal wall time per added execution:  (t_k - t_1) / (k - 1).
    This subtracts the fixed per-dispatch tunnel overhead (~80 ms) that
    would otherwise swamp the measurement.  Returns ns.
    """
    import time
    import jax
    from jax.experimental.shard_map import shard_map
    from jax.sharding import Mesh, NamedSharding, PartitionSpec
    from concourse import bass2jax, mybir as mb

    nc = _get_prog()
    bass2jax.install_neuronx_cc_hook()
    in_maps = _in_maps(inputs)
    partition_name = nc.partition_id_tensor.name if nc.partition_id_tensor else None
    in_names, out_names, out_avals, zero_outs = [], [], [], []
    for alloc in nc.m.functions[0].allocations:
        if not isinstance(alloc, mb.MemoryLocationSet):
            continue
        name = alloc.memorylocations[0].name
        if alloc.kind == "ExternalInput":
            if name != partition_name:
                in_names.append(name)
        elif alloc.kind == "ExternalOutput":
            shape = tuple(alloc.tensor_shape)
            dtype = mb.dt.np(alloc.dtype)
            out_avals.append(jax.core.ShapedArray(shape, dtype))
            out_names.append(name)
            zero_outs.append(np.zeros(shape, dtype))
    n_params, n_outs = len(in_names), len(out_avals)
    all_in = list(in_names) + list(out_names)
    if partition_name is not None:
        all_in.append(partition_name)

    def _body(*args):
        ins = list(args[:n_params])
        outs = list(args[n_params:])
        operands = ins + outs
        if partition_name is not None:
            operands = operands + [bass2jax.partition_id_tensor()]
        outs = list(bass2jax._bass_exec_p.bind(
            *operands, out_avals=tuple(out_avals), in_names=tuple(all_in),
            out_names=tuple(out_names), lowering_input_output_aliases=(),
            sim_require_finite=True, sim_require_nnan=True, nc=nc))
        return tuple(outs)

    devices = jax.devices()[:NCORES]
    mesh = Mesh(np.asarray(devices), ("core",))
    shard = NamedSharding(mesh, PartitionSpec("core"))
    dev_in = [jax.device_put(
        np.concatenate([np.asarray(in_maps[c][nm]) for c in range(NCORES)], axis=0),
        shard) for nm in in_names]
    zsh = [np.zeros((NCORES * z.shape[0], *z.shape[1:]), z.dtype) for z in zero_outs]

    f = jax.jit(
        shard_map(_body, mesh=mesh,
                  in_specs=(PartitionSpec("core"),) * (n_params + n_outs),
                  out_specs=(PartitionSpec("core"),) * n_outs, check_rep=False),
        keep_unused=True)

    def run_chain(k):
        outs = [jax.device_put(z, shard) for z in zsh]
        jax.block_until_ready(outs)
        jax.block_until_ready(dev_in)
        t0 = time.perf_counter()
        for _ in range(k):
            outs = list(f(*dev_in, *outs))
        jax.block_until_ready(outs)
        return time.perf_counter() - t0

    run_chain(1)  # warm compile
    t1 = min(run_chain(1) for _ in range(3))
    tk = min(run_chain(iters) for _ in range(3))
    return int((tk - t1) / (iters - 1) * 1e9)


def debug_run(inputs, core=0, n_layers=NL, ncores=1, phase=99):
    """Run the debug program; returns (y, t_rm_dump) for one core."""
    nc = _get_prog(debug=True, n_layers=n_layers, phase=phase)
    res = run_bass_kernel_spmd(nc, _in_maps(inputs)[:ncores], core_ids=list(range(ncores)))
    return res.results[core]["yc"], res.results[core]["dbg"]

